# revision 20
# baseline (speedup 1.0000x reference)
"""Causal dilated conv1d (K=3, dilation=2, N=128 channels) on Trainium2.

out[b,t,i] = sum_{j,k} x[b, t-2k, j] * weight[i,j,k] + bias[i]

Strategy (8-core SPMD, pure data parallel over batch; bf16 datapath):
  - each core handles 4 of the 32 batch rows; weight/bias replicated.
  - host interleaves the core's 4 rows channel-major with a 16-row zero
    head:  x4[tt, j, q] = x[b_q, tt-16, j]  (zeros for tt<16).  A DMA
    xbar transpose then loads a [2064, 512] DRAM slab directly as
    [128(j), 4(q), 2064(t)] in SBUF: 1 KB descriptors (4 rows per
    descriptor instead of 1) and the zero head doubles as causal pad,
    so the PE does no transpose-in and no edge special-casing.
  - taps: 3 accumulated bf16 matmuls per 512-wide PSUM window, moving
    operand = strip shifted by 2k columns.
  - ACT adds per-partition bias while copying PSUM->SBUF (bf16 out).
  - PE transposes the [i,t] result back to [t,i] using a stride-16
    moving operand so each output partition holds 16 consecutive t rows
    -> 4 KB contiguous store descriptors, issued on the idle GpSimd
    SWDGE queue so stores never block the xbar transpose stream.
  - output is bf16; host upconverts to fp32.
"""

import threading

import numpy as np
import ml_dtypes

import concourse.bass as bass  # noqa: F401  (bass types used via bacc/tile)
import concourse.mybir as mybir
import concourse.tile as tile
from concourse import bacc
from concourse.bass_utils import run_bass_kernel_spmd
from concourse.masks import make_identity

P = 128
KTAPS = 3
DIL = 2
NCORES = 8
B_FULL, T_FULL = 32, 8192
B_CORE = B_FULL // NCORES  # 4
HEAD = 16  # zero rows prepended on host (causal pad + keeps slabs aligned)

FP32 = mybir.dt.float32
BF16 = mybir.dt.bfloat16


def build(T=T_FULL, slabs=(1024, 2048, 2048, 2048, 1024)):
    """Build the per-core Bass module. Same NEFF runs SPMD on all 8 cores.

    Variable slab sizes: small first slab so the PE starts after ~2.5us
    of xbar streaming instead of ~9.5, small last slab so the restore/
    store pipeline drains quickly.
    """
    assert sum(slabs) == T
    nc = bacc.Bacc(
        "TRN2",
        target_bir_lowering=False,
        debug=False,
        enable_asserts=False,
        num_devices=NCORES,
    )
    Q = B_CORE  # interleaved batch rows per core
    x_d = nc.dram_tensor("x", [HEAD + T, Q, P], BF16, kind="ExternalInput")
    w_d = nc.dram_tensor("w", [P, KTAPS * P], BF16, kind="ExternalInput")
    b_d = nc.dram_tensor("b", [P, 1], FP32, kind="ExternalInput")
    o_d = nc.dram_tensor("o", [Q, T, P], BF16, kind="ExternalOutput")

    x_ap, o_ap = x_d.ap(), o_d.ap()
    SW = 512  # tap-matmul moving width (1 PSUM bank of fp32)

    with tile.TileContext(nc) as tc:
        with (
            tc.tile_pool(name="const", bufs=1) as cp,
            tc.tile_pool(name="strip", bufs=4) as sp,
            tc.tile_pool(name="oT", bufs=2) as otp,
            tc.tile_pool(name="oc", bufs=6) as ocp,
            tc.tile_pool(name="pacc", bufs=4, space="PSUM") as paccp,
            tc.tile_pool(name="pto", bufs=4, space="PSUM") as ptop,
        ):
            ident = cp.tile([P, P], FP32)
            make_identity(nc, ident)
            ident_bf = cp.tile([P, P], BF16)
            nc.vector.tensor_copy(ident_bf[:], ident[:])
            # const loads on the GpSimd SWDGE queue so the sync queue's first
            # instruction is slab 0's xbar transpose (shortens the head).
            w_sb = cp.tile([P, KTAPS * P], BF16)
            nc.gpsimd.dma_start(w_sb[:], w_d.ap())
            bias_sb = cp.tile([P, 1], FP32)
            nc.gpsimd.dma_start(bias_sb[:], b_d.ap())

            # one-(slab,row)-delayed transpose-out so the PE never stalls
            # waiting on ACT's PSUM->SBUF bias copies.
            pending = []  # fifo of (t0, chunk, q, oT)

            def emit_tout(t0_p, ch_p, q_p, oT_p):
                # column r + R*p of the row-q window holds t = t0 + p*R + r
                oTv = oT_p[:, q_p * ch_p : (q_p + 1) * ch_p].rearrange(
                    "n (p r) -> n r p", p=P
                )
                oc_full = ocp.tile([P, max(slabs)], BF16, tag="oc")
                oc = oc_full[:, :ch_p]
                for g in range(ch_p // SW):
                    pto = ptop.tile([P, SW], BF16, tag="pto")
                    for rr in range(4):
                        r = g * 4 + rr
                        nc.tensor.transpose(
                            pto[:, rr * P : (rr + 1) * P], oTv[:, r, :], ident_bf
                        )
                    nc.vector.tensor_copy(oc[:, g * SW : (g + 1) * SW], pto[:])
                # SWDGE on the idle GpSimd queue: stores never block the sync
                # queue's xbar transpose stream.
                nc.gpsimd.dma_start(
                    o_ap[q_p, t0_p : t0_p + ch_p, :].rearrange(
                        "(p r) i -> p (r i)", p=P
                    ),
                    oc[:],
                )

            t0 = 0
            for chunk in slabs:
                SLABT = chunk + HEAD
                # strip[j, q*SLABT + tt] = x[b_q, t0 + tt - HEAD, j].
                # A fresh tile per slab: no WAR/RAW aliasing between slabs,
                # so all slab transposes stream back-to-back on the xbar.
                strip_full = sp.tile([P, Q * (max(slabs) + HEAD)], BF16, tag="strip")
                strip = strip_full[:, : Q * SLABT]
                # xbar-transposed load of one slab, all 4 rows at once:
                # [chunk+16, 4*128] DRAM -> [128, 4, chunk+16] SBUF.
                nc.sync.dma_start_transpose(
                    strip.rearrange("p (q t) -> p q t", q=Q),
                    x_ap[t0 : t0 + SLABT, :, :],
                )
                oT_full = otp.tile([P, Q * max(slabs)], BF16, tag="oT")
                oT = oT_full[:, : Q * chunk]
                for q in range(Q):
                    base = q * SLABT + HEAD
                    for s in range(chunk // SW):
                        pacc = paccp.tile([P, SW], FP32, tag="pacc")
                        for k in range(KTAPS):
                            off = base + s * SW - DIL * k
                            nc.tensor.matmul(
                                pacc[:],
                                w_sb[:, k * P : (k + 1) * P],
                                strip[:, off : off + SW],
                                start=(k == 0),
                                stop=(k == KTAPS - 1),
                            )
                        nc.scalar.add(
                            oT[:, q * chunk + s * SW : q * chunk + (s + 1) * SW],
                            pacc[:],
                            bias_sb[:],
                        )
                    # delayed transpose-out: restore one pending (slab,row)
                    # from 2 rows ago while taps keep the PE dense.
                    if len(pending) >= 2:
                        emit_tout(*pending.pop(0))
                    pending.append((t0, chunk, q, oT))
                t0 += chunk
            for args in pending:
                emit_tout(*args)
    nc.compile()
    return nc


_cache = {}
_lock = threading.Lock()


def _get_nc():
    with _lock:
        if "nc" not in _cache:
            _cache["nc"] = build()
        return _cache["nc"]


def prep_inputs(x, weight, bias):
    # w_all[j, k*128 + i] = weight[i, j, k]
    w_all = np.ascontiguousarray(
        np.transpose(np.asarray(weight, np.float32), (1, 2, 0))
        .reshape(P, KTAPS * P)
        .astype(ml_dtypes.bfloat16)
    )
    b2 = np.ascontiguousarray(np.asarray(bias, np.float32).reshape(P, 1))
    xb = np.asarray(x, np.float32).astype(ml_dtypes.bfloat16)
    # per core: x4[tt, q, j] = x[b_q, tt-HEAD, j], 16 zero rows at the top
    # (q-major: the xbar fills transposed rows partition-first, so row
    # r = q*128+j lands at partition j, sub-slab q)
    xi = np.zeros((NCORES, HEAD + T_FULL, B_CORE, P), dtype=ml_dtypes.bfloat16)
    xg = xb.reshape(NCORES, B_CORE, T_FULL, P)
    xi[:, HEAD:, :, :] = np.swapaxes(xg, 1, 2)
    return xi, w_all, b2


def kernel(x, weight, bias, _trace=False):
    xi, w_all, b2 = prep_inputs(x, weight, bias)
    nc = _get_nc()
    in_maps = [
        {"x": np.ascontiguousarray(xi[c]), "w": w_all, "b": b2}
        for c in range(NCORES)
    ]
    res = run_bass_kernel_spmd(nc, in_maps, core_ids=list(range(NCORES)), trace=_trace)
    out = np.concatenate(
        [np.asarray(r["o"]).astype(np.float32) for r in res.results], axis=0
    )
    if _trace:
        kernel.last_results = res
    return out


# revision 22
# speedup vs baseline: 1.0465x; 1.0465x over previous
"""Causal dilated conv1d (K=3, dilation=2, N=128 channels) on Trainium2.

out[b,t,i] = sum_{j,k} x[b, t-2k, j] * weight[i,j,k] + bias[i]

Strategy (8-core SPMD, pure data parallel over batch; bf16 datapath):
  - each core handles 4 of the 32 batch rows; weight/bias replicated.
  - host interleaves the core's 4 rows channel-major with a 16-row zero
    head:  x4[tt, j, q] = x[b_q, tt-16, j]  (zeros for tt<16).  A DMA
    xbar transpose then loads a [2064, 512] DRAM slab directly as
    [128(j), 4(q), 2064(t)] in SBUF: 1 KB descriptors (4 rows per
    descriptor instead of 1) and the zero head doubles as causal pad,
    so the PE does no transpose-in and no edge special-casing.
  - taps: 3 accumulated bf16 matmuls per 512-wide PSUM window, moving
    operand = strip shifted by 2k columns.
  - ACT adds per-partition bias while copying PSUM->SBUF (bf16 out).
  - PE transposes the [i,t] result back to [t,i] using a stride-16
    moving operand so each output partition holds 16 consecutive t rows
    -> 4 KB contiguous store descriptors, issued on the idle GpSimd
    SWDGE queue so stores never block the xbar transpose stream.
  - output is bf16; host upconverts to fp32.
"""

import threading

import numpy as np
import ml_dtypes

import concourse.bass as bass  # noqa: F401  (bass types used via bacc/tile)
import concourse.mybir as mybir
import concourse.tile as tile
from concourse import bacc
from concourse.bass_utils import run_bass_kernel_spmd
from concourse.masks import make_identity

P = 128
KTAPS = 3
DIL = 2
NCORES = 8
B_FULL, T_FULL = 32, 8192
B_CORE = B_FULL // NCORES  # 4
HEAD = 16  # zero rows prepended on host (causal pad + keeps slabs aligned)

FP32 = mybir.dt.float32
BF16 = mybir.dt.bfloat16


def build(T=T_FULL, slabs=(1024, 2048, 2048, 2048, 1024)):
    """Build the per-core Bass module. Same NEFF runs SPMD on all 8 cores.

    Variable slab sizes: small first slab so the PE starts after ~2.5us
    of xbar streaming instead of ~9.5, small last slab so the restore/
    store pipeline drains quickly.
    """
    assert sum(slabs) == T
    nc = bacc.Bacc(
        "TRN2",
        target_bir_lowering=False,
        debug=False,
        enable_asserts=False,
        num_devices=NCORES,
    )
    Q = B_CORE  # interleaved batch rows per core
    x_d = nc.dram_tensor("x", [HEAD + T, Q, P], BF16, kind="ExternalInput")
    w_d = nc.dram_tensor("w", [P, KTAPS * P], BF16, kind="ExternalInput")
    b_d = nc.dram_tensor("b", [P, 1], FP32, kind="ExternalInput")
    o_d = nc.dram_tensor("o", [Q, T, P], BF16, kind="ExternalOutput")

    x_ap, o_ap = x_d.ap(), o_d.ap()
    SW = 512  # tap-matmul moving width (1 PSUM bank of fp32)

    with tile.TileContext(nc) as tc:
        with (
            tc.tile_pool(name="const", bufs=1) as cp,
            tc.tile_pool(name="strip", bufs=5) as sp,
            tc.tile_pool(name="oT", bufs=2) as otp,
            tc.tile_pool(name="oc", bufs=6) as ocp,
            tc.tile_pool(name="pacc", bufs=4, space="PSUM") as paccp,
            tc.tile_pool(name="pto", bufs=4, space="PSUM") as ptop,
        ):
            ident = cp.tile([P, P], FP32)
            make_identity(nc, ident)
            ident_bf = cp.tile([P, P], BF16)
            nc.vector.tensor_copy(ident_bf[:], ident[:])
            w_sb = cp.tile([P, KTAPS * P], BF16)
            nc.sync.dma_start(w_sb[:], w_d.ap())
            bias_sb = cp.tile([P, 1], FP32)
            nc.sync.dma_start(bias_sb[:], b_d.ap())

            # one-(slab,row)-delayed transpose-out so the PE never stalls
            # waiting on ACT's PSUM->SBUF bias copies.
            pending = []  # fifo of (t0, chunk, q, oT)

            def emit_tout(t0_p, ch_p, q_p, oT_p):
                # column r + R*p of the row-q window holds t = t0 + p*R + r
                oTv = oT_p[:, q_p * ch_p : (q_p + 1) * ch_p].rearrange(
                    "n (p r) -> n r p", p=P
                )
                oc_full = ocp.tile([P, max(slabs)], BF16, tag="oc")
                oc = oc_full[:, :ch_p]
                for g in range(ch_p // SW):
                    pto = ptop.tile([P, SW], BF16, tag="pto")
                    for rr in range(4):
                        r = g * 4 + rr
                        nc.tensor.transpose(
                            pto[:, rr * P : (rr + 1) * P], oTv[:, r, :], ident_bf
                        )
                    nc.vector.tensor_copy(oc[:, g * SW : (g + 1) * SW], pto[:])
                # SWDGE on the idle GpSimd queue: stores never block the sync
                # queue's xbar transpose stream.
                nc.gpsimd.dma_start(
                    o_ap[q_p, t0_p : t0_p + ch_p, :].rearrange(
                        "(p r) i -> p (r i)", p=P
                    ),
                    oc[:],
                )

            t0 = 0
            for chunk in slabs:
                SLABT = chunk + HEAD
                # strip[j, q*SLABT + tt] = x[b_q, t0 + tt - HEAD, j].
                # A fresh tile per slab: no WAR/RAW aliasing between slabs,
                # so all slab transposes stream back-to-back on the xbar.
                strip_full = sp.tile([P, Q * (max(slabs) + HEAD)], BF16, tag="strip")
                strip = strip_full[:, : Q * SLABT]
                # xbar-transposed load of one slab, all 4 rows at once:
                # [chunk+16, 4*128] DRAM -> [128, 4, chunk+16] SBUF.
                nc.sync.dma_start_transpose(
                    strip.rearrange("p (q t) -> p q t", q=Q),
                    x_ap[t0 : t0 + SLABT, :, :],
                )
                oT_full = otp.tile([P, Q * max(slabs)], BF16, tag="oT")
                oT = oT_full[:, : Q * chunk]
                for q in range(Q):
                    base = q * SLABT + HEAD
                    for s in range(chunk // SW):
                        pacc = paccp.tile([P, SW], FP32, tag="pacc")
                        for k in range(KTAPS):
                            off = base + s * SW - DIL * k
                            nc.tensor.matmul(
                                pacc[:],
                                w_sb[:, k * P : (k + 1) * P],
                                strip[:, off : off + SW],
                                start=(k == 0),
                                stop=(k == KTAPS - 1),
                            )
                        nc.scalar.add(
                            oT[:, q * chunk + s * SW : q * chunk + (s + 1) * SW],
                            pacc[:],
                            bias_sb[:],
                        )
                    # delayed transpose-out: restore one pending (slab,row)
                    # from 2 rows ago while taps keep the PE dense.
                    if len(pending) >= 2:
                        emit_tout(*pending.pop(0))
                    pending.append((t0, chunk, q, oT))
                t0 += chunk
            for args in pending:
                emit_tout(*args)
    nc.compile()
    return nc


_cache = {}
_lock = threading.Lock()


def _get_nc():
    with _lock:
        if "nc" not in _cache:
            _cache["nc"] = build()
        return _cache["nc"]


def prep_inputs(x, weight, bias):
    # w_all[j, k*128 + i] = weight[i, j, k]
    w_all = np.ascontiguousarray(
        np.transpose(np.asarray(weight, np.float32), (1, 2, 0))
        .reshape(P, KTAPS * P)
        .astype(ml_dtypes.bfloat16)
    )
    b2 = np.ascontiguousarray(np.asarray(bias, np.float32).reshape(P, 1))
    xb = np.asarray(x, np.float32).astype(ml_dtypes.bfloat16)
    # per core: x4[tt, q, j] = x[b_q, tt-HEAD, j], 16 zero rows at the top
    # (q-major: the xbar fills transposed rows partition-first, so row
    # r = q*128+j lands at partition j, sub-slab q)
    xi = np.zeros((NCORES, HEAD + T_FULL, B_CORE, P), dtype=ml_dtypes.bfloat16)
    xg = xb.reshape(NCORES, B_CORE, T_FULL, P)
    xi[:, HEAD:, :, :] = np.swapaxes(xg, 1, 2)
    return xi, w_all, b2


def kernel(x, weight, bias, _trace=False):
    xi, w_all, b2 = prep_inputs(x, weight, bias)
    nc = _get_nc()
    in_maps = [
        {"x": np.ascontiguousarray(xi[c]), "w": w_all, "b": b2}
        for c in range(NCORES)
    ]
    res = run_bass_kernel_spmd(nc, in_maps, core_ids=list(range(NCORES)), trace=_trace)
    out = np.concatenate(
        [np.asarray(r["o"]).astype(np.float32) for r in res.results], axis=0
    )
    if _trace:
        kernel.last_results = res
    return out


# revision 23
# speedup vs baseline: 1.1459x; 1.0950x over previous
"""Causal dilated conv1d (K=3, dilation=2, N=128 channels) on Trainium2.

out[b,t,i] = sum_{j,k} x[b, t-2k, j] * weight[i,j,k] + bias[i]

Strategy (8-core SPMD, pure data parallel over batch; bf16 datapath):
  - each core handles 4 of the 32 batch rows; weight/bias replicated.
  - host interleaves the core's 4 rows with a 16-row zero head:
    x4[tt, q, j] = x[b_q, tt-16, j]  (zeros for tt<16).  A DMA xbar
    transpose loads each [chunk+16, 512] DRAM slab directly as
    [128(j), 4(q), chunk+16(t)] in SBUF (transposed row r = q*128+j
    lands partition-first: partition j, sub-slab q), so one transfer
    feeds all 4 rows' strips, the PE does no transpose-in, and the
    zero head doubles as causal pad (no edge special-casing).
  - per-slab strip tiles (fresh pool tile per slab) keep the slab
    transposes free of false WAR deps: the tile framework tracks
    hazards at tile granularity, not byte ranges.
  - variable slab sizes (1024, 2048x3, 1024): small first slab starts
    the PE ~5us earlier, small last slab drains the tail faster.
  - taps: 3 accumulated bf16 matmuls per 512-wide PSUM window, moving
    operand = strip shifted by 2k columns.
  - ACT adds per-partition bias while copying PSUM->SBUF (bf16 out).
  - PE transposes the [i,t] result back to [t,i] using a stride-16
    moving operand so each output partition holds 16 consecutive t rows
    -> 4 KB contiguous store descriptors, issued on the idle GpSimd
    SWDGE queue so stores never block the xbar transpose stream.
  - output is bf16; host upconverts to fp32.
"""

import threading

import numpy as np
import ml_dtypes

import concourse.bass as bass  # noqa: F401  (bass types used via bacc/tile)
import concourse.mybir as mybir
import concourse.tile as tile
from concourse import bacc
from concourse.bass_utils import run_bass_kernel_spmd
from concourse.masks import make_identity

P = 128
KTAPS = 3
DIL = 2
NCORES = 8
B_FULL, T_FULL = 32, 8192
B_CORE = B_FULL // NCORES  # 4
HEAD = 16  # zero rows prepended on host (causal pad + keeps slabs aligned)

FP32 = mybir.dt.float32
BF16 = mybir.dt.bfloat16


def build(T=T_FULL, slabs=(1024, 2048, 2048, 2048, 1024)):
    """Build the per-core Bass module. Same NEFF runs SPMD on all 8 cores.

    Variable slab sizes: small first slab so the PE starts after ~2.5us
    of xbar streaming instead of ~9.5, small last slab so the restore/
    store pipeline drains quickly.
    """
    assert sum(slabs) == T
    nc = bacc.Bacc(
        "TRN2",
        target_bir_lowering=False,
        debug=False,
        enable_asserts=False,
        num_devices=NCORES,
    )
    Q = B_CORE  # interleaved batch rows per core
    x_d = nc.dram_tensor("x", [HEAD + T, Q, P], BF16, kind="ExternalInput")
    w_d = nc.dram_tensor("w", [P, KTAPS * P], BF16, kind="ExternalInput")
    b_d = nc.dram_tensor("b", [P, 1], FP32, kind="ExternalInput")
    o_d = nc.dram_tensor("o", [Q, T, P], BF16, kind="ExternalOutput")

    x_ap, o_ap = x_d.ap(), o_d.ap()
    SW = 512  # tap-matmul moving width (1 PSUM bank of fp32)

    with tile.TileContext(nc) as tc:
        with (
            tc.tile_pool(name="const", bufs=1) as cp,
            tc.tile_pool(name="strip", bufs=5) as sp,
            tc.tile_pool(name="oT", bufs=2) as otp,
            tc.tile_pool(name="oc", bufs=6) as ocp,
            tc.tile_pool(name="pacc", bufs=4, space="PSUM") as paccp,
            tc.tile_pool(name="pto", bufs=4, space="PSUM") as ptop,
        ):
            ident = cp.tile([P, P], FP32)
            make_identity(nc, ident)
            ident_bf = cp.tile([P, P], BF16)
            nc.vector.tensor_copy(ident_bf[:], ident[:])
            w_sb = cp.tile([P, KTAPS * P], BF16)
            nc.sync.dma_start(w_sb[:], w_d.ap())
            bias_sb = cp.tile([P, 1], FP32)
            nc.sync.dma_start(bias_sb[:], b_d.ap())

            # one-(slab,row)-delayed transpose-out so the PE never stalls
            # waiting on ACT's PSUM->SBUF bias copies.
            pending = []  # fifo of (t0, chunk, q, oT)

            def emit_tout(t0_p, ch_p, q_p, oT_p):
                # column r + R*p of the row-q window holds t = t0 + p*R + r
                oTv = oT_p[:, q_p * ch_p : (q_p + 1) * ch_p].rearrange(
                    "n (p r) -> n r p", p=P
                )
                oc_full = ocp.tile([P, max(slabs)], BF16, tag="oc")
                oc = oc_full[:, :ch_p]
                for g in range(ch_p // SW):
                    pto = ptop.tile([P, SW], BF16, tag="pto")
                    for rr in range(4):
                        r = g * 4 + rr
                        nc.tensor.transpose(
                            pto[:, rr * P : (rr + 1) * P], oTv[:, r, :], ident_bf
                        )
                    nc.vector.tensor_copy(oc[:, g * SW : (g + 1) * SW], pto[:])
                # SWDGE on the idle GpSimd queue: stores never block the sync
                # queue's xbar transpose stream.
                nc.gpsimd.dma_start(
                    o_ap[q_p, t0_p : t0_p + ch_p, :].rearrange(
                        "(p r) i -> p (r i)", p=P
                    ),
                    oc[:],
                )

            t0 = 0
            for chunk in slabs:
                SLABT = chunk + HEAD
                # strip[j, q*SLABT + tt] = x[b_q, t0 + tt - HEAD, j].
                # A fresh tile per slab: no WAR/RAW aliasing between slabs,
                # so all slab transposes stream back-to-back on the xbar.
                strip_full = sp.tile([P, Q * (max(slabs) + HEAD)], BF16, tag="strip")
                strip = strip_full[:, : Q * SLABT]
                # xbar-transposed load of one slab, all 4 rows at once:
                # [chunk+16, 4*128] DRAM -> [128, 4, chunk+16] SBUF.
                nc.sync.dma_start_transpose(
                    strip.rearrange("p (q t) -> p q t", q=Q),
                    x_ap[t0 : t0 + SLABT, :, :],
                )
                oT_full = otp.tile([P, Q * max(slabs)], BF16, tag="oT")
                oT = oT_full[:, : Q * chunk]
                for q in range(Q):
                    base = q * SLABT + HEAD
                    for s in range(chunk // SW):
                        pacc = paccp.tile([P, SW], FP32, tag="pacc")
                        for k in range(KTAPS):
                            off = base + s * SW - DIL * k
                            nc.tensor.matmul(
                                pacc[:],
                                w_sb[:, k * P : (k + 1) * P],
                                strip[:, off : off + SW],
                                start=(k == 0),
                                stop=(k == KTAPS - 1),
                            )
                        nc.scalar.add(
                            oT[:, q * chunk + s * SW : q * chunk + (s + 1) * SW],
                            pacc[:],
                            bias_sb[:],
                        )
                    # delayed transpose-out: restore one pending (slab,row)
                    # from 2 rows ago while taps keep the PE dense.
                    if len(pending) >= 2:
                        emit_tout(*pending.pop(0))
                    pending.append((t0, chunk, q, oT))
                t0 += chunk
            for args in pending:
                emit_tout(*args)
    nc.compile()
    return nc


_cache = {}
_lock = threading.Lock()


def _get_nc():
    with _lock:
        if "nc" not in _cache:
            _cache["nc"] = build()
        return _cache["nc"]


def prep_inputs(x, weight, bias):
    # w_all[j, k*128 + i] = weight[i, j, k]
    w_all = np.ascontiguousarray(
        np.transpose(np.asarray(weight, np.float32), (1, 2, 0))
        .reshape(P, KTAPS * P)
        .astype(ml_dtypes.bfloat16)
    )
    b2 = np.ascontiguousarray(np.asarray(bias, np.float32).reshape(P, 1))
    xb = np.asarray(x, np.float32).astype(ml_dtypes.bfloat16)
    # per core: x4[tt, q, j] = x[b_q, tt-HEAD, j], 16 zero rows at the top
    # (q-major: the xbar fills transposed rows partition-first, so row
    # r = q*128+j lands at partition j, sub-slab q)
    xi = np.zeros((NCORES, HEAD + T_FULL, B_CORE, P), dtype=ml_dtypes.bfloat16)
    xg = xb.reshape(NCORES, B_CORE, T_FULL, P)
    xi[:, HEAD:, :, :] = np.swapaxes(xg, 1, 2)
    return xi, w_all, b2


def kernel(x, weight, bias, _trace=False):
    xi, w_all, b2 = prep_inputs(x, weight, bias)
    nc = _get_nc()
    in_maps = [
        {"x": np.ascontiguousarray(xi[c]), "w": w_all, "b": b2}
        for c in range(NCORES)
    ]
    res = run_bass_kernel_spmd(nc, in_maps, core_ids=list(range(NCORES)), trace=_trace)
    out = np.concatenate(
        [np.asarray(r["o"]).astype(np.float32) for r in res.results], axis=0
    )
    if _trace:
        kernel.last_results = res
    return out
